# revision 10
# baseline (speedup 1.0000x reference)
"""BERT self-attention (B=4, S=2048, HID=768, 12 heads) on 8 NeuronCores.

Sharding: data-parallel over batch (4) x tensor-parallel over heads (2 groups
of 6 heads)  ->  8 cores, no cross-core communication.

Per-core device program (all matmuls in float32r = full-rate fp32 PE mode):
  A. hs^T via PE transposes; Q^T/K^T (head-dim-major) and V (seq-major)
     projections.  1/sqrt(64) is folded into Wq host-side (exact, power of 2).
  B. Per (head, 512-query block): scores^T = K^T.T @ Q^T chunks (k-major so
     softmax's additive mask and denominator fall out naturally), exp with the
     attention mask as the per-partition activation bias (no max subtraction:
     scores ~ N(0,1), fp32 exp cannot overflow), ctx^T = [V|1].T @ probs with
     an appended ones column producing the softmax denominator as row 64,
     PE-transpose back to seq-major, normalize with the reciprocal denominator.
"""

import numpy as np

import concourse.bacc as bacc
import concourse.mybir as mybir
import concourse.tile as tile
from concourse.bass_utils import run_bass_kernel_spmd
from concourse.masks import make_identity

F32 = mybir.dt.float32
F32R = mybir.dt.float32r
EXP = mybir.ActivationFunctionType.Exp

B = 4
S = 2048
HID = 768
NH_FULL = 12
HD = 64
NCORES = 8
NH = 6              # heads per core
D3 = NH * HD        # 384, per-core projection width
ST = S // 128       # 16 seq tiles
QB = 1024           # query block per exp (2 x 512 matmul chunks)
QC = 512            # fp32 moving-operand max per matmul
NQB = S // QB       # 2
KC = S // 128       # 16 key chunks

_nc_cache: dict = {}


def _build(ck: int):
    """Build the per-core program. ck = # of 128-row contraction chunks in the
    projection (6 plain, 7 when biases are folded in via an augmented row)."""
    nc = bacc.Bacc("TRN2", target_bir_lowering=False, debug=False)
    hs_d = nc.dram_tensor("hs", [S, ck * 128], F32, kind="ExternalInput")
    wq_d = nc.dram_tensor("wq", [ck * 128, D3], F32R, kind="ExternalInput")
    wk_d = nc.dram_tensor("wk", [ck * 128, D3], F32R, kind="ExternalInput")
    wv_d = nc.dram_tensor("wv", [ck * 128, D3], F32R, kind="ExternalInput")
    mask_d = nc.dram_tensor("mask", [128, KC], F32, kind="ExternalInput")
    out_d = nc.dram_tensor("out", [S, D3], F32, kind="ExternalOutput")

    with tile.TileContext(nc) as tc:
        with (
            tc.tile_pool(name="const", bufs=1) as constp,
            tc.tile_pool(name="qkpool", bufs=1) as qkp,
            tc.tile_pool(name="vpool", bufs=1) as vp,
        ):
            identity = constp.tile([128, 128], F32)
            make_identity(nc, identity)
            mask_sb = constp.tile([128, KC], F32)
            nc.sync.dma_start(mask_sb[:], mask_d[:])
            ones_sb = constp.tile([128, NH, 1], F32)
            nc.vector.memset(ones_sb[:], 1.0)
            zeros_sb = constp.tile([128, 63], F32)
            nc.vector.memset(zeros_sb[:], 0.0)

            qt = [qkp.tile([128, S], F32R, name=f"qt{m}") for m in range(3)]
            kt = [qkp.tile([128, S], F32R, name=f"kt{m}") for m in range(3)]
            # Per-pair zero-padded K^T stationaries: full [128,128] tile configs
            # keep the PE's background weight buffer active (partial row/col
            # configs serialize every weight load ~3x).
            kt_padA = qkp.tile([128, S], F32R, name="kt_padA")
            kt_padB = qkp.tile([128, S], F32R, name="kt_padB")
            nc.vector.memset(kt_padA[64:128, :].bitcast(F32), 0.0)
            nc.vector.memset(kt_padB[0:64, :].bitcast(F32), 0.0)
            # v stored flat: [v_h (64) | ones | ...] x 6 heads + 63-wide zero
            # tail, so each head's ctx stationary is a full-width 128-column
            # window starting at h*65 (overlap into the next head's columns
            # only produces garbage in unused output rows 65-127).
            VW = NH * (HD + 1) + 63  # 453
            v_sb = [vp.tile([128, VW], F32R, name=f"v{i}") for i in range(ST)]

            # ---------------- Phase A: transposes + projections ----------------
            with (
                tc.tile_pool(name="wpool", bufs=1) as wp,
                tc.tile_pool(name="hspool", bufs=3) as hsp,
                tc.tile_pool(name="hstpool", bufs=1) as hstp,
                tc.tile_pool(name="trps", bufs=2, space="PSUM") as trps,
                tc.tile_pool(name="mmps", bufs=2, space="PSUM") as mmps,
            ):
                wq_sb = wp.tile([128, ck, D3], F32R, name="wq_sb")
                wk_sb = wp.tile([128, ck, D3], F32R, name="wk_sb")
                wv_sb = wp.tile([128, ck, D3], F32R, name="wv_sb")
                nc.sync.dma_start(wq_sb[:], wq_d.ap().rearrange("(c p) n -> p c n", p=128))
                nc.sync.dma_start(wk_sb[:], wk_d.ap().rearrange("(c p) n -> p c n", p=128))
                nc.sync.dma_start(wv_sb[:], wv_d.ap().rearrange("(c p) n -> p c n", p=128))

                hsT = [hstp.tile([128, S], F32R, name=f"hsT{c}") for c in range(ck)]

                for st in range(ST):
                    hs_t = hsp.tile([128, ck * 128], F32, name="hs_t")
                    nc.sync.dma_start(hs_t[:], hs_d[st * 128:(st + 1) * 128, :])
                    for c in range(ck):
                        tp = trps.tile([128, 128], F32, name="tp")
                        nc.tensor.transpose(tp[:], hs_t[:, c * 128:(c + 1) * 128], identity[:])
                        nc.vector.tensor_copy(hsT[c][:, st * 128:(st + 1) * 128], tp[:])
                    # V projection for this seq tile (needs this tile's hsT cols)
                    vps = mmps.tile([128, D3], F32, name="vps")
                    for c in range(ck):
                        nc.tensor.matmul(
                            vps[:],
                            hsT[c][:, st * 128:(st + 1) * 128],
                            wv_sb[:, c, :],
                            start=(c == 0),
                            stop=(c == ck - 1),
                        )
                    v3 = v_sb[st][:, 0:NH * (HD + 1)].rearrange(
                        "p (h e) -> p h e", h=NH
                    )
                    nc.vector.tensor_copy(v3[:, :, HD:HD + 1], ones_sb[:])
                    nc.vector.tensor_copy(
                        v3[:, :, 0:HD],
                        vps.rearrange("p (h d) -> p h d", h=NH),
                    )
                    nc.vector.tensor_copy(v_sb[st][:, NH * (HD + 1):VW], zeros_sb[:])

                for m in range(3):
                    for n in range(S // QC):
                        for dst, w_sb in ((qt[m], wq_sb), (kt[m], wk_sb)):
                            ps = mmps.tile([128, QC], F32, name="ps")
                            for c in range(ck):
                                nc.tensor.matmul(
                                    ps[:],
                                    w_sb[:, c, m * 128:(m + 1) * 128],
                                    hsT[c][:, n * QC:(n + 1) * QC],
                                    start=(c == 0),
                                    stop=(c == ck - 1),
                                )
                            nc.vector.tensor_copy(dst[:, n * QC:(n + 1) * QC], ps[:])

            # ---------------- Phase B: attention ----------------
            with (
                tc.tile_pool(name="outpool", bufs=1) as outp,
                tc.tile_pool(name="prpool", bufs=1) as prp,
                tc.tile_pool(name="stps", bufs=2, space="PSUM") as stps,
                tc.tile_pool(name="ctxps", bufs=1, space="PSUM") as ctxps,
                tc.tile_pool(name="ctps", bufs=2, space="PSUM") as ctps,
                tc.tile_pool(name="ctxtpool", bufs=1) as ctxtp,
                tc.tile_pool(name="rdpool", bufs=4) as rdp,
            ):
                out_sb = [outp.tile([128, D3], F32, name=f"os{i}") for i in range(ST)]
                for hp in range(NH // 2):
                    # refresh the padded K^T stationaries for this head pair
                    nc.vector.tensor_copy(kt_padA[0:64, :], kt[hp][0:64, :])
                    nc.vector.tensor_copy(kt_padB[64:128, :], kt[hp][64:128, :])
                    for qb in range(NQB):
                        prs = []
                        # scores + exp for both heads of the pair
                        for hh in range(2):
                            ktp = kt_padA if hh == 0 else kt_padB
                            pr = prp.tile([128, KC, QB], F32R, name="pr")
                            prs.append(pr)
                            for kc in range(KC):
                                sps = stps.tile([128, QB], F32, name="sps")
                                for qc in range(QB // QC):
                                    nc.tensor.matmul(
                                        sps[:, qc * QC:(qc + 1) * QC],
                                        ktp[:, kc * 128:(kc + 1) * 128],
                                        qt[hp][:, qb * QB + qc * QC:qb * QB + (qc + 1) * QC],
                                    )
                                nc.scalar.activation(
                                    pr[:, kc, :], sps[:], EXP,
                                    bias=mask_sb[:, kc:kc + 1], scale=1.0,
                                )
                        # ctx^T accumulation + normalize/transpose tail
                        for hh in range(2):
                            h = hp * 2 + hh
                            pr = prs[hh]
                            cps = ctxps.tile([128, QB], F32, name="cps")
                            for kc in range(KC):
                                for qc in range(QB // QC):
                                    nc.tensor.matmul(
                                        cps[:, qc * QC:(qc + 1) * QC],
                                        v_sb[kc][:, h * (HD + 1):h * (HD + 1) + 128],
                                        pr[:, kc, qc * QC:(qc + 1) * QC],
                                        start=(kc == 0),
                                        stop=(kc == KC - 1),
                                    )
                            ctxt = ctxtp.tile([HD + 1, QB], F32, name="ctxt")
                            nc.vector.tensor_copy(ctxt[:], cps[0:HD + 1, :])
                            for qs in range(QB // 128):
                                sti = qb * (QB // 128) + qs
                                tp2 = ctps.tile([128, HD + 1], F32, name="tp2")
                                nc.tensor.transpose(
                                    tp2[:],
                                    ctxt[:, qs * 128:(qs + 1) * 128],
                                    identity[0:HD + 1, 0:HD + 1],
                                )
                                rd = rdp.tile([128, 1], F32, name="rd")
                                nc.vector.reciprocal(rd[:], tp2[:, HD:HD + 1])
                                nc.vector.tensor_scalar_mul(
                                    out_sb[sti][:, h * HD:(h + 1) * HD],
                                    tp2[:, 0:HD],
                                    rd[:],
                                )

                for st in range(ST):
                    nc.sync.dma_start(out_d[st * 128:(st + 1) * 128, :], out_sb[st][:])

    nc.compile()
    return nc


def _get_nc(ck: int):
    if ck not in _nc_cache:
        _nc_cache[ck] = _build(ck)
    return _nc_cache[ck]


def _prepare_in_maps(hidden_states, attention_mask, Wq, bq, Wk, bk, Wv, bv):
    hs = np.ascontiguousarray(np.asarray(hidden_states, dtype=np.float32))
    mask = np.asarray(attention_mask, dtype=np.float32).reshape(B, S)
    wq = np.asarray(Wq, dtype=np.float32) * np.float32(0.125)  # fold 1/sqrt(HD), exact
    wk = np.asarray(Wk, dtype=np.float32)
    wv = np.asarray(Wv, dtype=np.float32)
    bqs = np.asarray(bq, dtype=np.float32) * np.float32(0.125)
    bks = np.asarray(bk, dtype=np.float32)
    bvs = np.asarray(bv, dtype=np.float32)

    if bqs.any() or bks.any() or bvs.any():
        ck = 7
        pad = ck * 128 - (HID + 1)
        ones = np.ones((B, S, 1), np.float32)
        zer = np.zeros((B, S, pad), np.float32)
        hs_aug = np.ascontiguousarray(np.concatenate([hs, ones, zer], axis=2))
        def aug(w, b):
            return np.ascontiguousarray(
                np.concatenate([w, b[None, :], np.zeros((pad, HID), np.float32)], axis=0)
            )
        wq, wk, wv = aug(wq, bqs), aug(wk, bks), aug(wv, bvs)
        hs = hs_aug
    else:
        ck = 6

    in_maps = []
    for core in range(NCORES):
        b, hg = core // 2, core % 2
        cols = slice(hg * D3, (hg + 1) * D3)
        in_maps.append({
            "hs": np.ascontiguousarray(hs[b]),
            "wq": np.ascontiguousarray(wq[:, cols]),
            "wk": np.ascontiguousarray(wk[:, cols]),
            "wv": np.ascontiguousarray(wv[:, cols]),
            "mask": np.ascontiguousarray(mask[b].reshape(KC, 128).T),
        })
    return ck, in_maps


def run(hidden_states, attention_mask, Wq, bq, Wk, bk, Wv, bv, **rb_kwargs):
    """Shard, run on 8 cores, gather. Returns (output, BassKernelResults)."""
    ck, in_maps = _prepare_in_maps(
        hidden_states, attention_mask, Wq, bq, Wk, bk, Wv, bv
    )
    nc = _get_nc(ck)
    res = run_bass_kernel_spmd(nc, in_maps, core_ids=list(range(NCORES)), **rb_kwargs)
    out = np.empty((B, S, HID), dtype=np.float32)
    for core in range(NCORES):
        b, hg = core // 2, core % 2
        out[b, :, hg * D3:(hg + 1) * D3] = res.results[core]["out"]
    return out, res


def kernel(hidden_states, attention_mask, Wq, bq, Wk, bk, Wv, bv):
    out, _ = run(hidden_states, attention_mask, Wq, bq, Wk, bk, Wv, bv)
    return out


# revision 20
# speedup vs baseline: 1.0650x; 1.0650x over previous
"""BERT self-attention (B=4, S=2048, HID=768, 12 heads) on 8 NeuronCores.

Sharding: data-parallel over batch (4) x tensor-parallel over heads (2 groups
of 6 heads)  ->  8 cores, no cross-core communication.

Per-core device program (all matmuls in float32r = full-rate fp32 PE mode):
  A. hs^T via PE transposes; Q^T/K^T (head-dim-major) and V (seq-major)
     projections.  1/sqrt(64) is folded into Wq host-side (exact, power of 2).
  B. Per (head, 512-query block): scores^T = K^T.T @ Q^T chunks (k-major so
     softmax's additive mask and denominator fall out naturally), exp with the
     attention mask as the per-partition activation bias (no max subtraction:
     scores ~ N(0,1), fp32 exp cannot overflow), ctx^T = [V|1].T @ probs with
     an appended ones column producing the softmax denominator as row 64,
     PE-transpose back to seq-major, normalize with the reciprocal denominator.
"""

import numpy as np

import concourse.bacc as bacc
import concourse.mybir as mybir
import concourse.tile as tile
from concourse.bass_utils import run_bass_kernel_spmd
from concourse.masks import make_identity

F32 = mybir.dt.float32
F32R = mybir.dt.float32r
EXP = mybir.ActivationFunctionType.Exp

B = 4
S = 2048
HID = 768
NH_FULL = 12
HD = 64
NCORES = 8
NH = 6              # heads per core
D3 = NH * HD        # 384, per-core projection width
ST = S // 128       # 16 seq tiles
QB = 1024           # query block per exp (2 x 512 matmul chunks)
QC = 512            # fp32 moving-operand max per matmul
NQB = S // QB       # 2
KC = S // 128       # 16 key chunks

_nc_cache: dict = {}


def _build(ck: int):
    """Build the per-core program. ck = # of 128-row contraction chunks in the
    projection (6 plain, 7 when biases are folded in via an augmented row)."""
    nc = bacc.Bacc("TRN2", target_bir_lowering=False, debug=False)
    hs_d = nc.dram_tensor("hs", [S, ck * 128], F32, kind="ExternalInput")
    wq_d = nc.dram_tensor("wq", [ck * 128, D3], F32R, kind="ExternalInput")
    wk_d = nc.dram_tensor("wk", [ck * 128, D3], F32R, kind="ExternalInput")
    wv_d = nc.dram_tensor("wv", [ck * 128, D3], F32R, kind="ExternalInput")
    mask_d = nc.dram_tensor("mask", [128, KC], F32, kind="ExternalInput")
    out_d = nc.dram_tensor("out", [S, D3], F32, kind="ExternalOutput")

    with tile.TileContext(nc) as tc:
        with (
            tc.tile_pool(name="const", bufs=1) as constp,
            tc.tile_pool(name="qkpool", bufs=1) as qkp,
            tc.tile_pool(name="vpool", bufs=1) as vp,
        ):
            identity = constp.tile([128, 128], F32)
            make_identity(nc, identity)
            mask_sb = constp.tile([128, KC], F32)
            nc.sync.dma_start(mask_sb[:], mask_d[:])
            ones_sb = constp.tile([128, NH, 1], F32)
            nc.vector.memset(ones_sb[:], 1.0)
            zeros_sb = constp.tile([128, 63], F32)
            nc.vector.memset(zeros_sb[:], 0.0)

            qt = [qkp.tile([128, S], F32R, name=f"qt{m}") for m in range(3)]
            kt = [qkp.tile([128, S], F32R, name=f"kt{m}") for m in range(3)]
            # Per-pair zero-padded K^T stationaries: full [128,128] tile configs
            # keep the PE's background weight buffer active (partial row/col
            # configs serialize every weight load ~3x).
            kt_padA = qkp.tile([128, S], F32R, name="kt_padA")
            kt_padB = qkp.tile([128, S], F32R, name="kt_padB")
            nc.vector.memset(kt_padA[64:128, :].bitcast(F32), 0.0)
            nc.vector.memset(kt_padB[0:64, :].bitcast(F32), 0.0)
            # v stored flat: [v_h (64) | ones | ...] x 6 heads + 63-wide zero
            # tail, so each head's ctx stationary is a full-width 128-column
            # window starting at h*65 (overlap into the next head's columns
            # only produces garbage in unused output rows 65-127).
            VW = NH * (HD + 1) + 63  # 453
            v_sb = [vp.tile([128, VW], F32R, name=f"v{i}") for i in range(ST)]

            # ---------------- Phase A: transposes + projections ----------------
            with (
                tc.tile_pool(name="wpool", bufs=1) as wp,
                tc.tile_pool(name="hspool", bufs=3) as hsp,
                tc.tile_pool(name="hstpool", bufs=1) as hstp,
                tc.tile_pool(name="trps", bufs=4, space="PSUM") as trps,
                tc.tile_pool(name="mmps", bufs=2, space="PSUM") as mmps,
            ):
                wq_sb = wp.tile([128, ck, D3], F32R, name="wq_sb")
                wk_sb = wp.tile([128, ck, D3], F32R, name="wk_sb")
                wv_sb = wp.tile([128, ck, D3], F32R, name="wv_sb")

                hsT = [hstp.tile([128, S], F32R, name=f"hsT{c}") for c in range(ck)]

                for st in range(ST):
                    hs_t = hsp.tile([128, ck * 128], F32, name="hs_t")
                    nc.sync.dma_start(hs_t[:], hs_d[st * 128:(st + 1) * 128, :])
                    if st == 0:
                        # v-projection weights right after the first hs tile,
                        # chunk-by-chunk so the first V matmul starts early;
                        # q/k weights later (not needed until the qt/kt matmuls)
                        wv_r = wv_d.ap().rearrange("(c p) n -> p c n", p=128)
                        for c in range(ck):
                            nc.sync.dma_start(wv_sb[:, c, :], wv_r[:, c, :])
                    if st == 3:
                        nc.sync.dma_start(
                            wq_sb[:], wq_d.ap().rearrange("(c p) n -> p c n", p=128))
                        nc.sync.dma_start(
                            wk_sb[:], wk_d.ap().rearrange("(c p) n -> p c n", p=128))
                    for c in range(ck):
                        tp = trps.tile([128, 128], F32, name="tp")
                        nc.tensor.transpose(tp[:], hs_t[:, c * 128:(c + 1) * 128], identity[:])
                        # split PSUM->SBUF copies across DVE and the (idle
                        # during phase A) scalar engine
                        if c % 2 == 0:
                            nc.vector.tensor_copy(hsT[c][:, st * 128:(st + 1) * 128], tp[:])
                        else:
                            nc.scalar.copy(hsT[c][:, st * 128:(st + 1) * 128], tp[:])
                    # V projection for this seq tile (needs this tile's hsT cols)
                    vps = mmps.tile([128, D3], F32, name="vps")
                    for c in range(ck):
                        nc.tensor.matmul(
                            vps[:],
                            hsT[c][:, st * 128:(st + 1) * 128],
                            wv_sb[:, c, :],
                            start=(c == 0),
                            stop=(c == ck - 1),
                        )
                    v3 = v_sb[st][:, 0:NH * (HD + 1)].rearrange(
                        "p (h e) -> p h e", h=NH
                    )
                    nc.vector.tensor_copy(v3[:, :, HD:HD + 1], ones_sb[:])
                    nc.vector.tensor_copy(
                        v3[:, :, 0:HD],
                        vps.rearrange("p (h d) -> p h d", h=NH),
                    )
                    nc.vector.tensor_copy(v_sb[st][:, NH * (HD + 1):VW], zeros_sb[:])

                for m in range(3):
                    for n in range(S // QC):
                        for dst, w_sb in ((qt[m], wq_sb), (kt[m], wk_sb)):
                            ps = mmps.tile([128, QC], F32, name="ps")
                            for c in range(ck):
                                nc.tensor.matmul(
                                    ps[:],
                                    w_sb[:, c, m * 128:(m + 1) * 128],
                                    hsT[c][:, n * QC:(n + 1) * QC],
                                    start=(c == 0),
                                    stop=(c == ck - 1),
                                )
                            nc.vector.tensor_copy(dst[:, n * QC:(n + 1) * QC], ps[:])

            # ---------------- Phase B: attention ----------------
            with (
                tc.tile_pool(name="outpool", bufs=1) as outp,
                tc.tile_pool(name="prpool", bufs=1) as prp,
                tc.tile_pool(name="stps", bufs=2, space="PSUM") as stps,
                tc.tile_pool(name="ctxps", bufs=1, space="PSUM") as ctxps,
                tc.tile_pool(name="ctps", bufs=2, space="PSUM") as ctps,
                tc.tile_pool(name="ctxtpool", bufs=1) as ctxtp,
                tc.tile_pool(name="rdpool", bufs=4) as rdp,
            ):
                out_sb = [outp.tile([128, D3], F32, name=f"os{i}") for i in range(ST)]

                def emit_tail(h, qb, ctxt):
                    for qs in range(QB // 128):
                        sti = qb * (QB // 128) + qs
                        tp2 = ctps.tile([128, HD + 1], F32, name="tp2")
                        nc.tensor.transpose(
                            tp2[:],
                            ctxt[:, qs * 128:(qs + 1) * 128],
                            identity[0:HD + 1, 0:HD + 1],
                        )
                        rd = rdp.tile([128, 1], F32, name="rd")
                        nc.vector.reciprocal(rd[:], tp2[:, HD:HD + 1])
                        nc.vector.tensor_scalar_mul(
                            out_sb[sti][:, h * HD:(h + 1) * HD],
                            tp2[:, 0:HD],
                            rd[:],
                        )

                pending = []  # deferred (h, qb, ctxt) tails from the previous block
                # first head pair's padded K^T stationaries
                nc.vector.tensor_copy(kt_padA[0:64, :], kt[0][0:64, :])
                nc.vector.tensor_copy(kt_padB[64:128, :], kt[0][64:128, :])
                for hp in range(NH // 2):
                    for qb in range(NQB):
                        prs = []
                        # scores + exp for both heads of the pair
                        for hh in range(2):
                            ktp = kt_padA if hh == 0 else kt_padB
                            pr = prp.tile([128, KC, QB], F32R, name="pr")
                            prs.append(pr)
                            for kc in range(KC):
                                sps = stps.tile([128, QB], F32, name="sps")
                                for qc in range(QB // QC):
                                    nc.tensor.matmul(
                                        sps[:, qc * QC:(qc + 1) * QC],
                                        ktp[:, kc * 128:(kc + 1) * 128],
                                        qt[hp][:, qb * QB + qc * QC:qb * QB + (qc + 1) * QC],
                                    )
                                nc.scalar.activation(
                                    pr[:, kc, :], sps[:], EXP,
                                    bias=mask_sb[:, kc:kc + 1], scale=1.0,
                                )
                        # after the last scores that read the current pads,
                        # refresh them for the next head pair (overlaps ctx)
                        if qb == NQB - 1 and hp + 1 < NH // 2:
                            nc.vector.tensor_copy(kt_padA[0:64, :], kt[hp + 1][0:64, :])
                            nc.vector.tensor_copy(kt_padB[64:128, :], kt[hp + 1][64:128, :])
                        # previous block's tails overlap this block's scores/exp
                        for args in pending:
                            emit_tail(*args)
                        pending = []
                        # ctx^T accumulation; tails deferred to the next block
                        for hh in range(2):
                            h = hp * 2 + hh
                            pr = prs[hh]
                            cps = ctxps.tile([128, QB], F32, name="cps")
                            for kc in range(KC):
                                for qc in range(QB // QC):
                                    nc.tensor.matmul(
                                        cps[:, qc * QC:(qc + 1) * QC],
                                        v_sb[kc][:, h * (HD + 1):h * (HD + 1) + 128],
                                        pr[:, kc, qc * QC:(qc + 1) * QC],
                                        start=(kc == 0),
                                        stop=(kc == KC - 1),
                                    )
                            ctxt = ctxtp.tile([HD + 1, QB], F32, name="ctxt", bufs=2)
                            nc.vector.tensor_copy(ctxt[:], cps[0:HD + 1, :])
                            if hp == NH // 2 - 1 and qb == NQB - 1:
                                emit_tail(h, qb, ctxt)  # last block: no next scores to hide behind
                            else:
                                pending.append((h, qb, ctxt))
                for args in pending:
                    emit_tail(*args)

                for st in range(ST):
                    nc.sync.dma_start(out_d[st * 128:(st + 1) * 128, :], out_sb[st][:])

    nc.compile()
    return nc


def _get_nc(ck: int):
    if ck not in _nc_cache:
        _nc_cache[ck] = _build(ck)
    return _nc_cache[ck]


def _prepare_in_maps(hidden_states, attention_mask, Wq, bq, Wk, bk, Wv, bv):
    hs = np.ascontiguousarray(np.asarray(hidden_states, dtype=np.float32))
    mask = np.asarray(attention_mask, dtype=np.float32).reshape(B, S)
    wq = np.asarray(Wq, dtype=np.float32) * np.float32(0.125)  # fold 1/sqrt(HD), exact
    wk = np.asarray(Wk, dtype=np.float32)
    wv = np.asarray(Wv, dtype=np.float32)
    bqs = np.asarray(bq, dtype=np.float32) * np.float32(0.125)
    bks = np.asarray(bk, dtype=np.float32)
    bvs = np.asarray(bv, dtype=np.float32)

    if bqs.any() or bks.any() or bvs.any():
        ck = 7
        pad = ck * 128 - (HID + 1)
        ones = np.ones((B, S, 1), np.float32)
        zer = np.zeros((B, S, pad), np.float32)
        hs_aug = np.ascontiguousarray(np.concatenate([hs, ones, zer], axis=2))
        def aug(w, b):
            return np.ascontiguousarray(
                np.concatenate([w, b[None, :], np.zeros((pad, HID), np.float32)], axis=0)
            )
        wq, wk, wv = aug(wq, bqs), aug(wk, bks), aug(wv, bvs)
        hs = hs_aug
    else:
        ck = 6

    in_maps = []
    for core in range(NCORES):
        b, hg = core // 2, core % 2
        cols = slice(hg * D3, (hg + 1) * D3)
        in_maps.append({
            "hs": np.ascontiguousarray(hs[b]),
            "wq": np.ascontiguousarray(wq[:, cols]),
            "wk": np.ascontiguousarray(wk[:, cols]),
            "wv": np.ascontiguousarray(wv[:, cols]),
            "mask": np.ascontiguousarray(mask[b].reshape(KC, 128).T),
        })
    return ck, in_maps


def run(hidden_states, attention_mask, Wq, bq, Wk, bk, Wv, bv, **rb_kwargs):
    """Shard, run on 8 cores, gather. Returns (output, BassKernelResults)."""
    ck, in_maps = _prepare_in_maps(
        hidden_states, attention_mask, Wq, bq, Wk, bk, Wv, bv
    )
    nc = _get_nc(ck)
    res = run_bass_kernel_spmd(nc, in_maps, core_ids=list(range(NCORES)), **rb_kwargs)
    out = np.empty((B, S, HID), dtype=np.float32)
    for core in range(NCORES):
        b, hg = core // 2, core % 2
        out[b, :, hg * D3:(hg + 1) * D3] = res.results[core]["out"]
    return out, res


def kernel(hidden_states, attention_mask, Wq, bq, Wk, bk, Wv, bv):
    out, _ = run(hidden_states, attention_mask, Wq, bq, Wk, bk, Wv, bv)
    return out
